# revision 40
# baseline (speedup 1.0000x reference)
"""Trainium2 Bass kernel for nn_MultiHeadAttention_5033701670958.

Multi-head attention with Music-Transformer relative position bias
(skewing trick), causal mask, returning (out, attn).

Sharding: 8 cores = 2 batches x 4 head-pairs. Each core computes its
batch's projections restricted to its 2 heads, the full attention for
those heads (causal-pruned), and a partial output projection; the host
sums the 4 partials per batch (the "all-reduce after Wo") and
concatenates the attention maps.

Key device-side tricks:
  - All big matmuls run as fp32r (FP22 truncation, 1 cyc/row at N>=256).
  - The relative-position skew is a flat-memory reinterpretation: QE rows
    are written to a DRAM scratch at row stride S+1 and read back at row
    stride S, which lands QE[i, S-1-i+j] at Srel[i, j]. The scratch is
    fp8e4m3 (Srel is a tiny additive bias, sigma ~0.16) and both heads
    share one write + one read DMA per row block.
  - Srel is added into the QK PSUM with an identity matmul; exp with
    fused row-sum accumulation evacuates PSUM directly on the scalar
    engine, writing bf16 (exactly the operand the PV matmul needs).
  - Causal masking is affine_select on the Srel diagonal chunk (fill a
    large negative so exp underflows to 0, matching mask*-1e9).
  - 1/rowsum is exp(-ln(rowsum)) on ACT (the custom-DVE reciprocal fails
    this walrus build; InstReciprocal costs ~2us).
  - The PV matmul consumes UNNORMALIZED bf16 exp (PE-transposed 128x128
    blocks, causal-pruned); the output projection runs per head (K=64,
    row-packed) and the 1/rowsum scaling is applied after it, which is
    exact by linearity. The f32 attn output is exp * recip per chunk.
  - Upper-triangle attn blocks are never written: the PJRT run path
    donates zero-filled output buffers, so unwritten regions are 0.
  - Emission is a 3-stage software pipeline A(i+3)/B(i)/C(i-1) (skew
    prep / scores+softmax / transpose+PV+Wo) with projection chunks
    drip-fed between rounds, so no engine stalls on the skew roundtrip
    or the softmax chain. All DMA triggers stay on the SP queue -- on
    compute engines they block the sequencer on queue backpressure.
Measured (TimelineSim cost model, per core): 157 us vs a ~128 us DMA
roofline (~46 MB/core at ~360 GB/s); first correct version was 446 us.
"""

import os
import sys

import numpy as np

if "/opt/trn_rl_repo" not in sys.path:
    sys.path.insert(0, "/opt/trn_rl_repo")

P = 128
DH = 64
CH = 512
B, S, D, H = 2, 2048, 512, 8
HPC = H // 4  # heads per core = 2
N_CORES = 8


def _ap(tile_ap, offset_elems, pairs):
    """Build a raw AP over `tile_ap`'s tensor with an extra element offset."""
    import concourse.bass as bass

    return bass.AP(
        tensor=tile_ap.tensor,
        offset=tile_ap.offset + offset_elems,
        ap=[list(p) for p in pairs],
    )


def build_module(s=S, d=D):
    import concourse.bass as bass
    import concourse.mybir as mybir
    import concourse.tile as tile
    from concourse.masks import make_identity
    from contextlib import ExitStack

    f32 = mybir.dt.float32
    f32r = mybir.dt.float32r
    bf16 = mybir.dt.bfloat16
    fp8 = mybir.dt.float8e4
    AF = mybir.ActivationFunctionType

    nb = s // P     # q-row blocks
    ndc = d // P    # contraction chunks for projections
    nsc = s // CH   # 512-wide column chunks

    nc = bass.Bass()
    xq = nc.declare_dram_parameter("xq", [s, d], f32, isOutput=False)
    xk = nc.declare_dram_parameter("xk", [s, d], f32, isOutput=False)
    xv = nc.declare_dram_parameter("xv", [s, d], f32, isOutput=False)
    wq = nc.declare_dram_parameter("wq", [d, P], f32, isOutput=False)
    wk = nc.declare_dram_parameter("wk", [d, P], f32, isOutput=False)
    wv = nc.declare_dram_parameter("wv", [d, P], f32, isOutput=False)
    bq = nc.declare_dram_parameter("bq", [P, 1], f32, isOutput=False)
    bk = nc.declare_dram_parameter("bk", [P, 1], f32, isOutput=False)
    bv = nc.declare_dram_parameter("bv", [P, 1], f32, isOutput=False)
    wo = nc.declare_dram_parameter("wo", [P, d], f32, isOutput=False)
    et2 = nc.declare_dram_parameter("et2", [P, s], f32, isOutput=False)
    attn_o = nc.declare_dram_parameter("attn", [HPC, s, s], f32, isOutput=True)
    out_o = nc.declare_dram_parameter("out", [s, d], f32, isOutput=True)

    zrows = 132  # 128 written rows + slack for the skew-read overrun

    with tile.TileContext(nc) as tc:
        with ExitStack() as ctx:
            const = ctx.enter_context(tc.tile_pool(name="const", bufs=1))
            work = ctx.enter_context(tc.tile_pool(name="work", bufs=3))
            small = ctx.enter_context(tc.tile_pool(name="small", bufs=4))
            psmm = ctx.enter_context(tc.tile_pool(name="psmm", bufs=3, space="PSUM"))
            pst = ctx.enter_context(tc.tile_pool(name="pst", bufs=3, space="PSUM"))
            pspv = ctx.enter_context(tc.tile_pool(name="pspv", bufs=2, space="PSUM"))
            qestp = ctx.enter_context(tc.tile_pool(name="qestp", bufs=3))
            srelp = ctx.enter_context(tc.tile_pool(name="srelp", bufs=4))
            attnbp = ctx.enter_context(tc.tile_pool(name="attnbp", bufs=5))
            attp = ctx.enter_context(tc.tile_pool(name="attp", bufs=5))
            dram = ctx.enter_context(tc.tile_pool(name="dram", bufs=6, space="DRAM"))

            id_f32 = const.tile([P, P], f32)
            make_identity(nc, id_f32)

            id_bf16 = const.tile([P, P], bf16)
            make_identity(nc, id_bf16)
            id_fp8 = const.tile([P, P], fp8)
            make_identity(nc, id_fp8)

            wsb = {}
            bsb = {}
            xb0 = {}
            for nm0, x_d0 in (("q", xq), ("k", xk), ("v", xv)):
                xb = work.tile([P, CH // P, d], f32, tag="xload", name=f"xb{nm0}pre")
                nc.sync.dma_start(
                    xb[:], x_d0[0:CH, :].rearrange("(a p) d -> p a d", p=P)
                )
                xb0[nm0] = xb

            def load_round(dst_f32r, src_ap, tag):
                stg = work.tile(list(dst_f32r.shape), f32, tag="stg")
                nc.sync.dma_start(stg[:], src_ap)
                nc.any.tensor_copy(dst_f32r[:], stg[:])

            for nm, w_d, b_d in (("q", wq, bq), ("k", wk, bk), ("v", wv, bv)):
                wt = const.tile([P, ndc, P], f32r, tag=f"w{nm}")
                load_round(wt, w_d.rearrange("(c p) j -> p c j", p=P), "w")
                bt = const.tile([P, 1], f32, tag=f"b{nm}")
                nc.sync.dma_start(bt[:], b_d[:])
                wsb[nm] = wt
                bsb[nm] = bt
            et2_sb = const.tile([P, s], f32r)
            load_round(et2_sb, et2[:], "et")
            wo_sb = const.tile([P, d], f32r)
            load_round(wo_sb, wo[:], "wo")

            # ---- projections: qT/kT/vT [128 = 2 heads x 64dh, s] in per-chunk
            # tiles (fine-grained deps let the main loop start early) ----
            proj = {}
            for nm in ("q", "k", "v"):
                tdt = f32 if nm == "v" else f32r
                proj[nm] = [
                    const.tile([P, CH], tdt, tag=f"proj{nm}{sc}", name=f"proj{nm}{sc}")
                    for sc in range(nsc)
                ]
            def emit_proj_unit(nm, sc):
                x_d = {"q": xq, "k": xk, "v": xv}[nm]
                if True:
                    if sc == 0:
                        xb = xb0[nm]
                    else:
                        xb = work.tile(
                            [P, CH // P, d], f32, tag="xload", name=f"xb{nm}{sc}"
                        )
                        r0 = sc * CH
                        nc.sync.dma_start(
                            xb[:],
                            x_d[r0 : r0 + CH, :].rearrange("(a p) d -> p a d", p=P),
                        )
                    ps_proj = psmm.tile([P, CH], f32, tag="mm", name=f"psp{nm}{sc}")
                    for dc in range(ndc):
                        ps_tr = pst.tile([P, CH], f32, tag="tr", name=f"pst{nm}{sc}{dc}")
                        for sb in range(CH // P):
                            nc.tensor.transpose(
                                ps_tr[:, sb * P : (sb + 1) * P],
                                xb[:, sb, dc * P : (dc + 1) * P],
                                id_f32,
                            )
                        xT = small.tile([P, CH], f32r, tag="xT", name=f"xT{nm}{sc}{dc}")
                        if dc % 2 == 1:
                            nc.scalar.copy(xT[:], ps_tr[:])
                        else:
                            nc.vector.tensor_copy(xT[:], ps_tr[:])
                        nc.tensor.matmul(
                            ps_proj[:],
                            wsb[nm][:, dc, :],
                            xT[:],
                            start=(dc == 0),
                            stop=(dc == ndc - 1),
                        )
                    nc.scalar.activation(
                        proj[nm][sc][:],
                        ps_proj[:],
                        AF.Identity,
                        bias=bsb[nm][:],
                        scale=1.0,
                    )

            qT_ch, kT_ch, vT_ch = proj["q"], proj["k"], proj["v"]

            def qT_sl(hp, j0, j1):
                c = j0 // CH
                assert j1 <= (c + 1) * CH
                return qT_ch[c][hp : hp + DH, j0 - c * CH : j1 - c * CH]

            # ---- v natural blocks, bf16: v_bf[:, jb, :] = v[jb*128:(jb+1)*128, :] ----
            v_bf = const.tile([P, nb, P], bf16)

            def emit_vbf_group(g):
                nblk = min(4, nb - g * 4)
                ps_tr = pst.tile([P, CH], f32, tag="tr", name=f"pstv{g}")
                for b4 in range(nblk):
                    jb = g * 4 + b4
                    c = jb * P // CH
                    o = jb * P - c * CH
                    nc.tensor.transpose(
                        ps_tr[:, b4 * P : (b4 + 1) * P],
                        vT_ch[c][:, o : o + P],
                        id_f32,
                    )
                nc.vector.tensor_copy(
                    v_bf[:, g * 4 : g * 4 + nblk, :],
                    ps_tr[:, 0 : nblk * P].rearrange("p (a b) -> p a b", a=nblk),
                )

            # ---- main loop: 3-stage software pipeline ----
            # A(i): QE strips + skew roundtrip  ->  srel tiles
            # B(i): QK+inject, exp+rowsum, normalize, attn write, bf16 cast
            # C(i): PE-transpose attn, PV accumulate, output projection
            # Emitted as A(i+1), B(i), C(i-1) so the PE never waits on the
            # DRAM skew roundtrip or the softmax chain of the same block.
            def geom(i):
                i0 = i * P
                W = i0 + P  # exact causal width of block i
                return i0, W, (W + CH - 1) // CH

            def stage_a(i):
                i0, W, nch = geom(i)
                zlen = zrows * (s + 1)
                z = dram.tile([HPC * zlen], fp8, tag="z")
                qe_st = qestp.tile([P, HPC, s], fp8, tag="qest")
                for h in range(HPC):
                    hp = h * DH
                    qblk = qT_sl(hp, i0, i0 + P)
                    for c in range(nch):
                        w_c = min(CH, W - c * CH)
                        ps_qe = psmm.tile([P, CH], f32, tag="mm")
                        e0 = s - W + c * CH
                        nc.tensor.matmul(
                            ps_qe[:, 0:w_c],
                            qblk,
                            et2_sb[hp : hp + DH, e0 : e0 + w_c],
                        )
                        sl = slice(c * CH, c * CH + w_c)
                        if (c + h) % 2 == 1:
                            nc.scalar.copy(qe_st[:, h, sl], ps_qe[:, 0:w_c])
                        else:
                            nc.vector.tensor_copy(qe_st[:, h, sl], ps_qe[:, 0:w_c])
                # both heads in one skew write / one skew read
                nc.sync.dma_start(
                    _ap(z, (s + 1 - W), [[s + 1, P], [zlen, HPC], [1, W]]),
                    qe_st[:, :, 0:W],
                )
                srel2 = srelp.tile([P, HPC, s], fp8, tag="srel")
                nc.sync.dma_start(
                    srel2[:, :, 0:W],
                    _ap(z, s - i0, [[s, P], [zlen, HPC], [1, W]]),
                )
                srels = []
                wa = min(CH, W)
                for h in range(HPC):
                    nc.gpsimd.affine_select(
                        out=srel2[:, h, W - wa : W],
                        in_=srel2[:, h, W - wa : W],
                        pattern=[[-1, wa]],
                        base=i0 - (W - wa),
                        channel_multiplier=1,
                        compare_op=mybir.AluOpType.is_ge,
                        fill=-240.0,
                    )
                    srels.append(srel2[:, h, :])
                return srels

            def stage_b(i, srels):
                i0, W, nch = geom(i)
                attnbs = []
                for h in range(HPC):
                    hp = h * DH
                    qblk = qT_sl(hp, i0, i0 + P)
                    srel = srels[h]
                    live = (i + 1) * P
                    # exp lands directly in bf16: it is both the PV operand
                    # (unnormalized) and, scaled by 1/rowsum, the attn output
                    attnb = attnbp.tile([P, s], bf16, tag="attnb")
                    acc = small.tile([P, 4], f32, tag="acc")
                    for c in range(nch):
                        w_c = min(CH, live - c * CH)
                        sl = slice(c * CH, c * CH + w_c)
                        ps_s = psmm.tile([P, CH], f32, tag="mm")
                        nc.tensor.matmul(
                            ps_s[:, 0:w_c],
                            qblk,
                            kT_ch[c][hp : hp + DH, 0:w_c],
                            start=True,
                            stop=False,
                        )
                        nc.tensor.matmul(
                            ps_s[:, 0:w_c],
                            id_fp8[:],
                            srel[:, sl],
                            start=False,
                            stop=True,
                        )
                        nc.scalar.activation(
                            attnb[:, sl],
                            ps_s[:, 0:w_c],
                            AF.Exp,
                            scale=0.125,
                            accum_out=acc[:, c : c + 1],
                        )
                    red = small.tile([P, 1], f32, tag="red")
                    if nch > 1:
                        nc.vector.tensor_reduce(
                            red[:],
                            acc[:, 0:nch],
                            axis=mybir.AxisListType.X,
                            op=mybir.AluOpType.add,
                        )
                    else:
                        nc.vector.tensor_copy(red[:], acc[:, 0:1])
                    # 1/rowsum as exp(-ln(rowsum)) on ACT: the custom-DVE
                    # reciprocal fails this walrus build and InstReciprocal is
                    # ~2us; two tiny table ops land in the same error class as
                    # the softmax exp itself.
                    lnr = small.tile([P, 1], f32, tag="lnr")
                    nc.scalar.activation(lnr[:], red[:], AF.Ln)
                    recip = small.tile([P, 1], f32, tag="recip")
                    nc.scalar.activation(recip[:], lnr[:], AF.Exp, scale=-1.0)

                    attnf = work.tile([P, s], f32, tag="attnf")
                    for c in range(nch):
                        w_c = min(CH, live - c * CH)
                        sl = slice(c * CH, c * CH + w_c)
                        if (c + h) % 2 == 0:
                            nc.vector.tensor_scalar_mul(
                                attnf[:, sl], attnb[:, sl], recip[:]
                            )
                        else:
                            nc.gpsimd.tensor_scalar_mul(
                                attnf[:, sl], attnb[:, sl], recip[:]
                            )
                    nc.sync.dma_start(
                        attn_o[h, i0 : i0 + P, 0:live], attnf[:, 0:live]
                    )
                    attnbs.append((attnb, recip))
                return attnbs

            def stage_c(i, attnbs):
                i0, W, nch = geom(i)
                ps_pv = pspv.tile([P, P], f32, tag="pv")
                for h in range(HPC):
                    hp = h * DH
                    attnb, _ = attnbs[h]
                    ngrp = (i + 4) // 4
                    attnTs = []
                    for g in range(ngrp):
                        nblk = min(4, i + 1 - g * 4)
                        ps_t = pst.tile([P, CH], bf16, tag="tr")
                        for b4 in range(nblk):
                            jb = g * 4 + b4
                            nc.tensor.transpose(
                                ps_t[:, b4 * P : (b4 + 1) * P],
                                attnb[:, jb * P : (jb + 1) * P],
                                id_bf16,
                            )
                        attnT = attp.tile([P, CH], bf16, tag="attnT")
                        if g % 4 == 3:
                            nc.scalar.copy(
                                attnT[:, 0 : nblk * P], ps_t[:, 0 : nblk * P]
                            )
                        else:
                            nc.vector.tensor_copy(
                                attnT[:, 0 : nblk * P], ps_t[:, 0 : nblk * P]
                            )
                        attnTs.append((attnT, nblk))
                    for g, (attnT, nblk) in enumerate(attnTs):
                        for b4 in range(nblk):
                            jb = g * 4 + b4
                            nc.tensor.matmul(
                                ps_pv[hp : hp + DH, :],
                                v_bf[:, jb, hp : hp + DH],
                                attnT[:, b4 * P : (b4 + 1) * P],
                                start=(jb == 0),
                                stop=(jb == i),
                                tile_position=(0, hp),
                                skip_group_check=True,
                            )
                # output projection: per-head (K=64, row-packed), then the
                # softmax normalization is applied per head and summed
                outT = small.tile([P, P], f32r, tag="outT")
                nc.vector.tensor_copy(outT[:], ps_pv[:])
                ps_o0 = psmm.tile([P, CH], f32, tag="mm")
                ps_o1 = psmm.tile([P, CH], f32, tag="mm")
                nc.tensor.matmul(ps_o0[:, 0:d], outT[0:DH, :], wo_sb[0:DH, :])
                nc.tensor.matmul(ps_o1[:, 0:d], outT[DH:P, :], wo_sb[DH:P, :])
                tmp0 = small.tile([P, d], f32, tag="tmp0")
                tmp1 = small.tile([P, d], f32, tag="tmp1")
                nc.vector.tensor_scalar_mul(tmp0[:], ps_o0[:, 0:d], attnbs[0][1][:])
                nc.scalar.mul(tmp1[:], ps_o1[:, 0:d], attnbs[1][1][:])
                osb = small.tile([P, d], f32, tag="osb")
                nc.gpsimd.tensor_add(osb[:], tmp0[:], tmp1[:])
                nc.sync.dma_start(out_o[i0 : i0 + P, :], osb[:])

            for nm in ("q", "k", "v"):
                emit_proj_unit(nm, 0)
            emit_vbf_group(0)
            # (tensor, chunk) units spread one per round so projection bursts
            # don't monopolize the PE mid-pipeline
            sched = {}
            for c in range(1, nsc):
                sched.setdefault(max(0, 4 * c - 6), []).append(("q", c))
                sched.setdefault(max(0, 4 * c - 5), []).append(("k", c))
                sched.setdefault(max(0, 4 * c - 4), []).append(("v", c))
            # Round order: blocks 2..nb-1 then 0,1 — the heaviest C stages land
            # mid-pipeline against light B stages and the tail drains on the
            # smallest blocks (chunk drip deadlines still hold for this order).
            R = list(range(1, nb)) + [0] if nb > 4 else list(range(nb))
            srel_q = {R[k]: stage_a(R[k]) for k in range(min(3, nb))}
            attnb_q = {}
            for r in range(nb):
                i = R[r]
                for nm, c in sched.get(r, ()):
                    emit_proj_unit(nm, c)
                    if nm == "v":
                        emit_vbf_group(c)
                if r + 3 < nb:
                    srel_q[R[r + 3]] = stage_a(R[r + 3])
                attnb_q[i] = stage_b(i, srel_q.pop(i))
                if r - 1 >= 0:
                    j = R[r - 1]
                    stage_c(j, attnb_q.pop(j))
            stage_c(R[nb - 1], attnb_q.pop(R[nb - 1]))

    return nc


def split_excess_waits(nc, limit=1):
    """The walrus build in this container supports only `limit` sync-wait
    commands per instruction; move excess on_wait entries to preceding NoOps."""
    import concourse.mybir as mybir

    n_fixed = 0
    for f in nc.m.functions:
        for blk in f.blocks:
            new_list = []
            for inst in blk.instructions:
                si = getattr(inst, "sync_info", None)
                if si is not None and si.on_wait and len(si.on_wait) > limit:
                    waits = list(si.on_wait)
                    excess, keep = waits[:-limit], waits[-limit:]
                    for j in range(0, len(excess), limit):
                        chunk = excess[j : j + limit]
                        new_list.append(
                            mybir.InstNoOp(
                                name=f"waitfix-{n_fixed}-{j}",
                                engine=inst.engine,
                                sync_info=mybir.SyncInfo(
                                    on_wait=list(chunk), on_update=[]
                                ),
                                bass_nofuse=True,
                            )
                        )
                    si.on_wait = keep
                    n_fixed += 1
                new_list.append(inst)
            blk.instructions = new_list
    return n_fixed


def shard_inputs(v, k, q, Wq, bq, Wk, bk, Wv, bv, Wo, E, s=S):
    """Build the 8 per-core input maps."""
    et2 = np.concatenate([E.T, E.T], axis=0).astype(np.float32)  # [128, s]
    et2 = np.ascontiguousarray(et2)
    in_maps = []
    for c in range(N_CORES):
        b = c // 4
        h0 = (c % 4) * HPC  # first head index
        cols = slice(h0 * DH, (h0 + HPC) * DH)
        in_maps.append(
            {
                "xq": np.ascontiguousarray(q[b]),
                "xk": np.ascontiguousarray(k[b]),
                "xv": np.ascontiguousarray(v[b]),
                "wq": np.ascontiguousarray(Wq[:, cols]),
                "wk": np.ascontiguousarray(Wk[:, cols]),
                "wv": np.ascontiguousarray(Wv[:, cols]),
                "bq": np.ascontiguousarray(bq[cols].reshape(P, 1)),
                "bk": np.ascontiguousarray(bk[cols].reshape(P, 1)),
                "bv": np.ascontiguousarray(bv[cols].reshape(P, 1)),
                "wo": np.ascontiguousarray(Wo[cols, :]),
                "et2": et2,
            }
        )
    return in_maps


def gather_outputs(results, bo, s=S, d=D):
    out = np.zeros((B, s, d), np.float32)
    attn = np.empty((B, H, s, s), np.float32)
    for c in range(N_CORES):
        b = c // 4
        h0 = (c % 4) * HPC
        out[b] += results[c]["out"]
        attn[b, h0 : h0 + HPC] = results[c]["attn"]
    out += bo.reshape(1, 1, d)
    return out, attn


_NC_CACHE = {}


def kernel(v, k, q, mask, Wq, bq, Wk, bk, Wv, bv, Wo, bo, E):
    v = np.asarray(v, np.float32)
    k = np.asarray(k, np.float32)
    q = np.asarray(q, np.float32)
    mask = np.asarray(mask, np.float32)
    Wq, bq_ = np.asarray(Wq, np.float32), np.asarray(bq, np.float32)
    Wk, bk_ = np.asarray(Wk, np.float32), np.asarray(bk, np.float32)
    Wv, bv_ = np.asarray(Wv, np.float32), np.asarray(bv, np.float32)
    Wo, bo_ = np.asarray(Wo, np.float32), np.asarray(bo, np.float32)
    E = np.asarray(E, np.float32)
    s = q.shape[1]

    causal = np.array_equal(
        mask[0, 0], np.triu(np.ones((s, s), np.float32), k=1)
    )
    if not causal:
        # Fallback (not expected for this problem's inputs): exact numpy.
        import warnings

        warnings.warn("non-causal mask; using host fallback")
        return _numpy_reference(v, k, q, mask, Wq, bq_, Wk, bk_, Wv, bv_, Wo, bo_, E)

    from concourse.bass_utils import run_bass_kernel_spmd

    if s not in _NC_CACHE:
        nc_new = build_module(s=s)
        split_excess_waits(nc_new)
        _NC_CACHE[s] = nc_new
    nc = _NC_CACHE[s]

    in_maps = shard_inputs(v, k, q, Wq, bq_, Wk, bk_, Wv, bv_, Wo, E, s=s)
    res = run_bass_kernel_spmd(nc, in_maps, core_ids=list(range(N_CORES)))
    out, attn = gather_outputs(res.results, bo_, s=s)
    return out, attn


def _numpy_reference(v, k, q, mask, Wq, bq, Wk, bk, Wv, bv, Wo, bo, E):
    b, s, d = q.shape
    h, dh = H, DH
    max_seq = E.shape[0]

    def split(x):
        return x.reshape(b, s, h, dh).transpose(0, 2, 1, 3)

    qh = split(q @ Wq + bq)
    kh = split(k @ Wk + bk)
    vh = split(v @ Wv + bv)
    logits = np.einsum("bhqd,bhkd->bhqk", qh, kh)
    Eq = E[max(max_seq - s, 0) :, :]
    m = Eq.shape[0]
    QE = np.einsum("bhld,md->bhlm", qh, Eq)
    rows = np.arange(s)[:, None]
    cols = np.arange(m)[None, :]
    QE = QE * (cols >= (m - 1 - rows)).astype(QE.dtype)
    padded = np.pad(QE, ((0, 0), (0, 0), (0, 0), (1, 0)))
    Srel = padded.reshape(b, h, m + 1, s)[:, :, 1:, :]
    logits = (logits + Srel) / np.sqrt(np.float32(dh))
    logits = logits + mask * -1e9
    logits -= logits.max(-1, keepdims=True)
    attn = np.exp(logits)
    attn /= attn.sum(-1, keepdims=True)
    out = np.einsum("bhqk,bhkd->bhqd", attn, vh)
    out = out.transpose(0, 2, 1, 3).reshape(b, s, d)
    return (out @ Wo + bo).astype(np.float32), attn.astype(np.float32)


# revision 41
# speedup vs baseline: 1.0311x; 1.0311x over previous
"""Trainium2 Bass kernel for nn_MultiHeadAttention_5033701670958.

Multi-head attention with Music-Transformer relative position bias
(skewing trick), causal mask, returning (out, attn).

Sharding: 8 cores = 2 batches x 4 head-pairs. Each core computes its
batch's projections restricted to its 2 heads, the full attention for
those heads (causal-pruned), and a partial output projection; the host
sums the 4 partials per batch (the "all-reduce after Wo") and
concatenates the attention maps.

Key device-side tricks:
  - All big matmuls run as fp32r (FP22 truncation, 1 cyc/row at N>=256).
  - The relative-position skew is a flat-memory reinterpretation: QE rows
    are written to a DRAM scratch at row stride S+1 and read back at row
    stride S, which lands QE[i, S-1-i+j] at Srel[i, j]. The scratch is
    fp8e4m3 (Srel is a tiny additive bias, sigma ~0.16) and both heads
    share one write + one read DMA per row block.
  - Srel is added into the QK PSUM with an identity matmul; exp with
    fused row-sum accumulation evacuates PSUM directly on the scalar
    engine, writing bf16 (exactly the operand the PV matmul needs).
  - Causal masking is affine_select on the Srel diagonal chunk (fill a
    large negative so exp underflows to 0, matching mask*-1e9).
  - 1/rowsum is exp(-ln(rowsum)) on ACT (the custom-DVE reciprocal fails
    this walrus build; InstReciprocal costs ~2us).
  - The PV matmul consumes UNNORMALIZED bf16 exp (PE-transposed 128x128
    blocks, causal-pruned); the output projection runs per head (K=64,
    row-packed) and the 1/rowsum scaling is applied after it, which is
    exact by linearity. The f32 attn output is exp * recip per chunk.
  - Upper-triangle attn blocks are never written: the PJRT run path
    donates zero-filled output buffers, so unwritten regions are 0.
  - Emission is a 3-stage software pipeline A(i+3)/B(i)/C(i-1) (skew
    prep / scores+softmax / transpose+PV+Wo) with projection chunks
    drip-fed between rounds, so no engine stalls on the skew roundtrip
    or the softmax chain. All DMA triggers stay on the SP queue -- on
    compute engines they block the sequencer on queue backpressure.
Measured (TimelineSim cost model, per core): 157 us vs a ~128 us DMA
roofline (~46 MB/core at ~360 GB/s); first correct version was 446 us.
"""

import os
import sys

import numpy as np

if "/opt/trn_rl_repo" not in sys.path:
    sys.path.insert(0, "/opt/trn_rl_repo")

P = 128
DH = 64
CH = 512
B, S, D, H = 2, 2048, 512, 8
HPC = H // 4  # heads per core = 2
N_CORES = 8


def _ap(tile_ap, offset_elems, pairs):
    """Build a raw AP over `tile_ap`'s tensor with an extra element offset."""
    import concourse.bass as bass

    return bass.AP(
        tensor=tile_ap.tensor,
        offset=tile_ap.offset + offset_elems,
        ap=[list(p) for p in pairs],
    )


def build_module(s=S, d=D):
    import concourse.bass as bass
    import concourse.mybir as mybir
    import concourse.tile as tile
    from concourse.masks import make_identity
    from contextlib import ExitStack

    f32 = mybir.dt.float32
    f32r = mybir.dt.float32r
    bf16 = mybir.dt.bfloat16
    fp8 = mybir.dt.float8e4
    AF = mybir.ActivationFunctionType

    nb = s // P     # q-row blocks
    ndc = d // P    # contraction chunks for projections
    nsc = s // CH   # 512-wide column chunks

    nc = bass.Bass()
    xq = nc.declare_dram_parameter("xq", [s, d], f32, isOutput=False)
    xk = nc.declare_dram_parameter("xk", [s, d], f32, isOutput=False)
    xv = nc.declare_dram_parameter("xv", [s, d], f32, isOutput=False)
    wq = nc.declare_dram_parameter("wq", [d, P], f32, isOutput=False)
    wk = nc.declare_dram_parameter("wk", [d, P], f32, isOutput=False)
    wv = nc.declare_dram_parameter("wv", [d, P], f32, isOutput=False)
    bq = nc.declare_dram_parameter("bq", [P, 1], f32, isOutput=False)
    bk = nc.declare_dram_parameter("bk", [P, 1], f32, isOutput=False)
    bv = nc.declare_dram_parameter("bv", [P, 1], f32, isOutput=False)
    wo = nc.declare_dram_parameter("wo", [P, d], f32, isOutput=False)
    et2 = nc.declare_dram_parameter("et2", [P, s], f32, isOutput=False)
    attn_o = nc.declare_dram_parameter("attn", [HPC, s, s], f32, isOutput=True)
    out_o = nc.declare_dram_parameter("out", [s, d], f32, isOutput=True)

    zrows = 132  # 128 written rows + slack for the skew-read overrun

    with tile.TileContext(nc) as tc:
        with ExitStack() as ctx:
            const = ctx.enter_context(tc.tile_pool(name="const", bufs=1))
            work = ctx.enter_context(tc.tile_pool(name="work", bufs=3))
            small = ctx.enter_context(tc.tile_pool(name="small", bufs=4))
            psmm = ctx.enter_context(tc.tile_pool(name="psmm", bufs=3, space="PSUM"))
            pst = ctx.enter_context(tc.tile_pool(name="pst", bufs=3, space="PSUM"))
            pspv = ctx.enter_context(tc.tile_pool(name="pspv", bufs=2, space="PSUM"))
            qestp = ctx.enter_context(tc.tile_pool(name="qestp", bufs=3))
            srelp = ctx.enter_context(tc.tile_pool(name="srelp", bufs=4))
            attnbp = ctx.enter_context(tc.tile_pool(name="attnbp", bufs=5))
            attp = ctx.enter_context(tc.tile_pool(name="attp", bufs=3))
            dram = ctx.enter_context(tc.tile_pool(name="dram", bufs=6, space="DRAM"))

            id_f32 = const.tile([P, P], f32)
            make_identity(nc, id_f32)

            id_bf16 = const.tile([P, P], bf16)
            make_identity(nc, id_bf16)
            id_fp8 = const.tile([P, P], fp8)
            make_identity(nc, id_fp8)

            wsb = {}
            bsb = {}
            xb0 = {}
            for nm0, x_d0 in (("q", xq), ("k", xk), ("v", xv)):
                xb = work.tile([P, CH // P, d], f32, tag="xload", name=f"xb{nm0}pre")
                nc.sync.dma_start(
                    xb[:], x_d0[0:CH, :].rearrange("(a p) d -> p a d", p=P)
                )
                xb0[nm0] = xb

            def load_round(dst_f32r, src_ap, tag):
                stg = work.tile(list(dst_f32r.shape), f32, tag="stg")
                nc.sync.dma_start(stg[:], src_ap)
                nc.any.tensor_copy(dst_f32r[:], stg[:])

            for nm, w_d, b_d in (("q", wq, bq), ("k", wk, bk), ("v", wv, bv)):
                wt = const.tile([P, ndc, P], f32r, tag=f"w{nm}")
                load_round(wt, w_d.rearrange("(c p) j -> p c j", p=P), "w")
                bt = const.tile([P, 1], f32, tag=f"b{nm}")
                nc.sync.dma_start(bt[:], b_d[:])
                wsb[nm] = wt
                bsb[nm] = bt
            et2_sb = const.tile([P, s], f32r)
            load_round(et2_sb, et2[:], "et")
            wo_sb = const.tile([P, d], f32r)
            load_round(wo_sb, wo[:], "wo")

            # ---- projections: qT/kT/vT [128 = 2 heads x 64dh, s] in per-chunk
            # tiles (fine-grained deps let the main loop start early) ----
            proj = {}
            for nm in ("q", "k", "v"):
                tdt = f32 if nm == "v" else f32r
                proj[nm] = [
                    const.tile([P, CH], tdt, tag=f"proj{nm}{sc}", name=f"proj{nm}{sc}")
                    for sc in range(nsc)
                ]
            def emit_proj_unit(nm, sc):
                x_d = {"q": xq, "k": xk, "v": xv}[nm]
                if True:
                    if sc == 0:
                        xb = xb0[nm]
                    else:
                        xb = work.tile(
                            [P, CH // P, d], f32, tag="xload", name=f"xb{nm}{sc}"
                        )
                        r0 = sc * CH
                        nc.sync.dma_start(
                            xb[:],
                            x_d[r0 : r0 + CH, :].rearrange("(a p) d -> p a d", p=P),
                        )
                    ps_proj = psmm.tile([P, CH], f32, tag="mm", name=f"psp{nm}{sc}")
                    for dc in range(ndc):
                        ps_tr = pst.tile([P, CH], f32, tag="tr", name=f"pst{nm}{sc}{dc}")
                        for sb in range(CH // P):
                            nc.tensor.transpose(
                                ps_tr[:, sb * P : (sb + 1) * P],
                                xb[:, sb, dc * P : (dc + 1) * P],
                                id_f32,
                            )
                        xT = small.tile([P, CH], f32r, tag="xT", name=f"xT{nm}{sc}{dc}")
                        if dc % 2 == 1:
                            nc.scalar.copy(xT[:], ps_tr[:])
                        else:
                            nc.vector.tensor_copy(xT[:], ps_tr[:])
                        nc.tensor.matmul(
                            ps_proj[:],
                            wsb[nm][:, dc, :],
                            xT[:],
                            start=(dc == 0),
                            stop=(dc == ndc - 1),
                        )
                    nc.scalar.activation(
                        proj[nm][sc][:],
                        ps_proj[:],
                        AF.Identity,
                        bias=bsb[nm][:],
                        scale=1.0,
                    )

            qT_ch, kT_ch, vT_ch = proj["q"], proj["k"], proj["v"]

            def qT_sl(hp, j0, j1):
                c = j0 // CH
                assert j1 <= (c + 1) * CH
                return qT_ch[c][hp : hp + DH, j0 - c * CH : j1 - c * CH]

            # ---- v natural blocks, bf16: v_bf[:, jb, :] = v[jb*128:(jb+1)*128, :] ----
            v_bf = const.tile([P, nb, P], bf16)

            def emit_vbf_group(g):
                nblk = min(4, nb - g * 4)
                ps_tr = pst.tile([P, CH], f32, tag="tr", name=f"pstv{g}")
                for b4 in range(nblk):
                    jb = g * 4 + b4
                    c = jb * P // CH
                    o = jb * P - c * CH
                    nc.tensor.transpose(
                        ps_tr[:, b4 * P : (b4 + 1) * P],
                        vT_ch[c][:, o : o + P],
                        id_f32,
                    )
                nc.vector.tensor_copy(
                    v_bf[:, g * 4 : g * 4 + nblk, :],
                    ps_tr[:, 0 : nblk * P].rearrange("p (a b) -> p a b", a=nblk),
                )

            # ---- main loop: 3-stage software pipeline ----
            # A(i): QE strips + skew roundtrip  ->  srel tiles
            # B(i): QK+inject, exp+rowsum, normalize, attn write, bf16 cast
            # C(i): PE-transpose attn, PV accumulate, output projection
            # Emitted as A(i+1), B(i), C(i-1) so the PE never waits on the
            # DRAM skew roundtrip or the softmax chain of the same block.
            def geom(i):
                i0 = i * P
                W = i0 + P  # exact causal width of block i
                return i0, W, (W + CH - 1) // CH

            def stage_a(i):
                i0, W, nch = geom(i)
                zlen = zrows * (s + 1)
                z = dram.tile([HPC * zlen], fp8, tag="z")
                qe_st = qestp.tile([P, HPC, s], fp8, tag="qest")
                for h in range(HPC):
                    hp = h * DH
                    qblk = qT_sl(hp, i0, i0 + P)
                    for c in range(nch):
                        w_c = min(CH, W - c * CH)
                        ps_qe = psmm.tile([P, CH], f32, tag="mm")
                        e0 = s - W + c * CH
                        nc.tensor.matmul(
                            ps_qe[:, 0:w_c],
                            qblk,
                            et2_sb[hp : hp + DH, e0 : e0 + w_c],
                        )
                        sl = slice(c * CH, c * CH + w_c)
                        if (c + h) % 2 == 1:
                            nc.scalar.copy(qe_st[:, h, sl], ps_qe[:, 0:w_c])
                        else:
                            nc.vector.tensor_copy(qe_st[:, h, sl], ps_qe[:, 0:w_c])
                # both heads in one skew write / one skew read
                nc.sync.dma_start(
                    _ap(z, (s + 1 - W), [[s + 1, P], [zlen, HPC], [1, W]]),
                    qe_st[:, :, 0:W],
                )
                srel2 = srelp.tile([P, HPC, s], fp8, tag="srel")
                nc.sync.dma_start(
                    srel2[:, :, 0:W],
                    _ap(z, s - i0, [[s, P], [zlen, HPC], [1, W]]),
                )
                srels = []
                wa = min(CH, W)
                for h in range(HPC):
                    nc.gpsimd.affine_select(
                        out=srel2[:, h, W - wa : W],
                        in_=srel2[:, h, W - wa : W],
                        pattern=[[-1, wa]],
                        base=i0 - (W - wa),
                        channel_multiplier=1,
                        compare_op=mybir.AluOpType.is_ge,
                        fill=-240.0,
                    )
                    srels.append(srel2[:, h, :])
                return srels

            def stage_b(i, srels):
                i0, W, nch = geom(i)
                attnbs = []
                for h in range(HPC):
                    hp = h * DH
                    qblk = qT_sl(hp, i0, i0 + P)
                    srel = srels[h]
                    live = (i + 1) * P
                    # exp lands directly in bf16: it is both the PV operand
                    # (unnormalized) and, scaled by 1/rowsum, the attn output
                    attnb = attnbp.tile([P, s], bf16, tag="attnb")
                    acc = small.tile([P, 4], f32, tag="acc")
                    for c in range(nch):
                        w_c = min(CH, live - c * CH)
                        sl = slice(c * CH, c * CH + w_c)
                        ps_s = psmm.tile([P, CH], f32, tag="mm")
                        nc.tensor.matmul(
                            ps_s[:, 0:w_c],
                            qblk,
                            kT_ch[c][hp : hp + DH, 0:w_c],
                            start=True,
                            stop=False,
                        )
                        nc.tensor.matmul(
                            ps_s[:, 0:w_c],
                            id_fp8[:],
                            srel[:, sl],
                            start=False,
                            stop=True,
                        )
                        nc.scalar.activation(
                            attnb[:, sl],
                            ps_s[:, 0:w_c],
                            AF.Exp,
                            scale=0.125,
                            accum_out=acc[:, c : c + 1],
                        )
                    red = small.tile([P, 1], f32, tag="red")
                    if nch > 1:
                        nc.vector.tensor_reduce(
                            red[:],
                            acc[:, 0:nch],
                            axis=mybir.AxisListType.X,
                            op=mybir.AluOpType.add,
                        )
                    else:
                        nc.vector.tensor_copy(red[:], acc[:, 0:1])
                    # 1/rowsum as exp(-ln(rowsum)) on ACT: the custom-DVE
                    # reciprocal fails this walrus build and InstReciprocal is
                    # ~2us; two tiny table ops land in the same error class as
                    # the softmax exp itself.
                    lnr = small.tile([P, 1], f32, tag="lnr")
                    nc.scalar.activation(lnr[:], red[:], AF.Ln)
                    recip = small.tile([P, 1], f32, tag="recip")
                    nc.scalar.activation(recip[:], lnr[:], AF.Exp, scale=-1.0)

                    attnf = work.tile([P, s], f32, tag="attnf")
                    for c in range(nch):
                        w_c = min(CH, live - c * CH)
                        sl = slice(c * CH, c * CH + w_c)
                        if (c + h) % 2 == 0:
                            nc.vector.tensor_scalar_mul(
                                attnf[:, sl], attnb[:, sl], recip[:]
                            )
                        else:
                            nc.gpsimd.tensor_scalar_mul(
                                attnf[:, sl], attnb[:, sl], recip[:]
                            )
                    nc.sync.dma_start(
                        attn_o[h, i0 : i0 + P, 0:live], attnf[:, 0:live]
                    )
                    attnbs.append((attnb, recip))
                return attnbs

            def stage_c(i, attnbs):
                i0, W, nch = geom(i)
                ps_pv = pspv.tile([P, P], f32, tag="pv")
                for h in range(HPC):
                    hp = h * DH
                    attnb, _ = attnbs[h]
                    ngrp = (i + 8) // 8
                    attnTs = []
                    for g in range(ngrp):
                        nblk = min(8, i + 1 - g * 8)
                        ps_t = pst.tile([P, 2 * CH], bf16, tag="tr")
                        for b4 in range(nblk):
                            jb = g * 8 + b4
                            nc.tensor.transpose(
                                ps_t[:, b4 * P : (b4 + 1) * P],
                                attnb[:, jb * P : (jb + 1) * P],
                                id_bf16,
                            )
                        attnT = attp.tile([P, 2 * CH], bf16, tag="attnT")
                        if g % 4 == 3:
                            nc.scalar.copy(
                                attnT[:, 0 : nblk * P], ps_t[:, 0 : nblk * P]
                            )
                        else:
                            nc.vector.tensor_copy(
                                attnT[:, 0 : nblk * P], ps_t[:, 0 : nblk * P]
                            )
                        attnTs.append((attnT, nblk))
                    for g, (attnT, nblk) in enumerate(attnTs):
                        for b4 in range(nblk):
                            jb = g * 8 + b4
                            nc.tensor.matmul(
                                ps_pv[hp : hp + DH, :],
                                v_bf[:, jb, hp : hp + DH],
                                attnT[:, b4 * P : (b4 + 1) * P],
                                start=(jb == 0),
                                stop=(jb == i),
                                tile_position=(0, hp),
                                skip_group_check=True,
                            )
                # output projection: per-head (K=64, row-packed), then the
                # softmax normalization is applied per head and summed
                outT = small.tile([P, P], f32r, tag="outT")
                nc.vector.tensor_copy(outT[:], ps_pv[:])
                ps_o0 = psmm.tile([P, CH], f32, tag="mm")
                ps_o1 = psmm.tile([P, CH], f32, tag="mm")
                nc.tensor.matmul(ps_o0[:, 0:d], outT[0:DH, :], wo_sb[0:DH, :])
                nc.tensor.matmul(ps_o1[:, 0:d], outT[DH:P, :], wo_sb[DH:P, :])
                tmp0 = small.tile([P, d], f32, tag="tmp0")
                tmp1 = small.tile([P, d], f32, tag="tmp1")
                nc.vector.tensor_scalar_mul(tmp0[:], ps_o0[:, 0:d], attnbs[0][1][:])
                nc.scalar.mul(tmp1[:], ps_o1[:, 0:d], attnbs[1][1][:])
                osb = small.tile([P, d], f32, tag="osb")
                nc.gpsimd.tensor_add(osb[:], tmp0[:], tmp1[:])
                nc.sync.dma_start(out_o[i0 : i0 + P, :], osb[:])

            for nm in ("q", "k", "v"):
                emit_proj_unit(nm, 0)
            emit_vbf_group(0)
            # (tensor, chunk) units spread one per round so projection bursts
            # don't monopolize the PE mid-pipeline
            sched = {}
            for c in range(1, nsc):
                sched.setdefault(max(0, 4 * c - 6), []).append(("q", c))
                sched.setdefault(max(0, 4 * c - 5), []).append(("k", c))
                sched.setdefault(max(0, 4 * c - 4), []).append(("v", c))
            # Round order: blocks 2..nb-1 then 0,1 — the heaviest C stages land
            # mid-pipeline against light B stages and the tail drains on the
            # smallest blocks (chunk drip deadlines still hold for this order).
            R = list(range(1, nb)) + [0] if nb > 4 else list(range(nb))
            srel_q = {R[k]: stage_a(R[k]) for k in range(min(3, nb))}
            attnb_q = {}
            for r in range(nb):
                i = R[r]
                for nm, c in sched.get(r, ()):
                    emit_proj_unit(nm, c)
                    if nm == "v":
                        emit_vbf_group(c)
                if r + 3 < nb:
                    srel_q[R[r + 3]] = stage_a(R[r + 3])
                attnb_q[i] = stage_b(i, srel_q.pop(i))
                if r - 1 >= 0:
                    j = R[r - 1]
                    stage_c(j, attnb_q.pop(j))
            stage_c(R[nb - 1], attnb_q.pop(R[nb - 1]))

    return nc


def split_excess_waits(nc, limit=1):
    """The walrus build in this container supports only `limit` sync-wait
    commands per instruction; move excess on_wait entries to preceding NoOps."""
    import concourse.mybir as mybir

    n_fixed = 0
    for f in nc.m.functions:
        for blk in f.blocks:
            new_list = []
            for inst in blk.instructions:
                si = getattr(inst, "sync_info", None)
                if si is not None and si.on_wait and len(si.on_wait) > limit:
                    waits = list(si.on_wait)
                    excess, keep = waits[:-limit], waits[-limit:]
                    for j in range(0, len(excess), limit):
                        chunk = excess[j : j + limit]
                        new_list.append(
                            mybir.InstNoOp(
                                name=f"waitfix-{n_fixed}-{j}",
                                engine=inst.engine,
                                sync_info=mybir.SyncInfo(
                                    on_wait=list(chunk), on_update=[]
                                ),
                                bass_nofuse=True,
                            )
                        )
                    si.on_wait = keep
                    n_fixed += 1
                new_list.append(inst)
            blk.instructions = new_list
    return n_fixed


def shard_inputs(v, k, q, Wq, bq, Wk, bk, Wv, bv, Wo, E, s=S):
    """Build the 8 per-core input maps."""
    et2 = np.concatenate([E.T, E.T], axis=0).astype(np.float32)  # [128, s]
    et2 = np.ascontiguousarray(et2)
    in_maps = []
    for c in range(N_CORES):
        b = c // 4
        h0 = (c % 4) * HPC  # first head index
        cols = slice(h0 * DH, (h0 + HPC) * DH)
        in_maps.append(
            {
                "xq": np.ascontiguousarray(q[b]),
                "xk": np.ascontiguousarray(k[b]),
                "xv": np.ascontiguousarray(v[b]),
                "wq": np.ascontiguousarray(Wq[:, cols]),
                "wk": np.ascontiguousarray(Wk[:, cols]),
                "wv": np.ascontiguousarray(Wv[:, cols]),
                "bq": np.ascontiguousarray(bq[cols].reshape(P, 1)),
                "bk": np.ascontiguousarray(bk[cols].reshape(P, 1)),
                "bv": np.ascontiguousarray(bv[cols].reshape(P, 1)),
                "wo": np.ascontiguousarray(Wo[cols, :]),
                "et2": et2,
            }
        )
    return in_maps


def gather_outputs(results, bo, s=S, d=D):
    out = np.zeros((B, s, d), np.float32)
    attn = np.empty((B, H, s, s), np.float32)
    for c in range(N_CORES):
        b = c // 4
        h0 = (c % 4) * HPC
        out[b] += results[c]["out"]
        attn[b, h0 : h0 + HPC] = results[c]["attn"]
    out += bo.reshape(1, 1, d)
    return out, attn


_NC_CACHE = {}


def kernel(v, k, q, mask, Wq, bq, Wk, bk, Wv, bv, Wo, bo, E):
    v = np.asarray(v, np.float32)
    k = np.asarray(k, np.float32)
    q = np.asarray(q, np.float32)
    mask = np.asarray(mask, np.float32)
    Wq, bq_ = np.asarray(Wq, np.float32), np.asarray(bq, np.float32)
    Wk, bk_ = np.asarray(Wk, np.float32), np.asarray(bk, np.float32)
    Wv, bv_ = np.asarray(Wv, np.float32), np.asarray(bv, np.float32)
    Wo, bo_ = np.asarray(Wo, np.float32), np.asarray(bo, np.float32)
    E = np.asarray(E, np.float32)
    s = q.shape[1]

    causal = np.array_equal(
        mask[0, 0], np.triu(np.ones((s, s), np.float32), k=1)
    )
    if not causal:
        # Fallback (not expected for this problem's inputs): exact numpy.
        import warnings

        warnings.warn("non-causal mask; using host fallback")
        return _numpy_reference(v, k, q, mask, Wq, bq_, Wk, bk_, Wv, bv_, Wo, bo_, E)

    from concourse.bass_utils import run_bass_kernel_spmd

    if s not in _NC_CACHE:
        nc_new = build_module(s=s)
        split_excess_waits(nc_new)
        _NC_CACHE[s] = nc_new
    nc = _NC_CACHE[s]

    in_maps = shard_inputs(v, k, q, Wq, bq_, Wk, bk_, Wv, bv_, Wo, E, s=s)
    res = run_bass_kernel_spmd(nc, in_maps, core_ids=list(range(N_CORES)))
    out, attn = gather_outputs(res.results, bo_, s=s)
    return out, attn


def _numpy_reference(v, k, q, mask, Wq, bq, Wk, bk, Wv, bv, Wo, bo, E):
    b, s, d = q.shape
    h, dh = H, DH
    max_seq = E.shape[0]

    def split(x):
        return x.reshape(b, s, h, dh).transpose(0, 2, 1, 3)

    qh = split(q @ Wq + bq)
    kh = split(k @ Wk + bk)
    vh = split(v @ Wv + bv)
    logits = np.einsum("bhqd,bhkd->bhqk", qh, kh)
    Eq = E[max(max_seq - s, 0) :, :]
    m = Eq.shape[0]
    QE = np.einsum("bhld,md->bhlm", qh, Eq)
    rows = np.arange(s)[:, None]
    cols = np.arange(m)[None, :]
    QE = QE * (cols >= (m - 1 - rows)).astype(QE.dtype)
    padded = np.pad(QE, ((0, 0), (0, 0), (0, 0), (1, 0)))
    Srel = padded.reshape(b, h, m + 1, s)[:, :, 1:, :]
    logits = (logits + Srel) / np.sqrt(np.float32(dh))
    logits = logits + mask * -1e9
    logits -= logits.max(-1, keepdims=True)
    attn = np.exp(logits)
    attn /= attn.sum(-1, keepdims=True)
    out = np.einsum("bhqk,bhkd->bhqd", attn, vh)
    out = out.transpose(0, 2, 1, 3).reshape(b, s, d)
    return (out @ Wo + bo).astype(np.float32), attn.astype(np.float32)
